# revision 1
# baseline (speedup 1.0000x reference)
"""DANet PAM attention (B=8, Cin=48, N=3136) on 8 Trainium2 NeuronCores.

Sharding: data-parallel over batch — one batch per core, zero communication.

Per-core program (one batch, all fp32):
  x2 = x[b] viewed as [48, 3136]; augmented with a ones row -> xa [49, 3136].
  q = wqT.T @ xa  [6, N]        (bias folded in via the ones row)
  k = wkT.T @ xa  [6, N]
  vT chunks = xa[:, jc].T @ wvT  [128, 65]  (col 64 of wvT = ones -> vT col 64
                                             is all-ones, used to get Z)
  Energy is computed TRANSPOSED: eT[j, i] = k[:,j]^T q[:,i], j on partitions,
  so that the V-weighted sum needs no on-chip transpose and the softmax
  denominator Z[i] = sum_j exp(eT[j,i]) falls out of the same matmul as U via
  vT's ones column:
      U[c, i] (+ Z at row 64) = sum_j vT[j, c] * exp(eT[j, i])
  Softmax max-subtraction is skipped: |energy| <~ 30, exp fits fp32 easily and
  the ratio exp(e)/Z is mathematically identical.
  out = gamma * U / Z + x2  (gamma baked in as the PE-broadcast constant).

PSUM budget (8 banks): 2 x 3 banks ping-pong for energy tiles (wide grouped
ACT exp ops cut the 352-cycle per-op overhead), 2 x 1 bank for U accumulator /
Z-broadcast.
"""

from contextlib import ExitStack

import numpy as np

F32 = None  # set on first build (concourse imported lazily)

B, C, T, H, W = 8, 3, 16, 56, 56
CIN = 48         # C*T
CA = 49          # augmented input channels (ones row)
CV = 65          # vT width: cols 0-47 = channels, col 64 = ones (32-aligned Z row)
CQ = 6
N = H * W        # 3136
IT = 448         # i-tile width (free dim per PSUM bank)
NIT = N // IT    # 7
JT = 128
NJT = (N + JT - 1) // JT   # 25 (24 full + one 64-row chunk)
GS = 3           # j-chunks per exp group

_CACHE = {}


def _build_nc(gamma: float):
    import concourse.bacc as bacc
    import concourse.mybir as mybir
    import concourse.tile as tile

    f32 = mybir.dt.float32
    nc = bacc.Bacc("TRN2", target_bir_lowering=False, debug=False)

    xa_d = nc.declare_dram_parameter("xa", [CA, N], f32, isOutput=False).ap()
    wq_d = nc.declare_dram_parameter("wqT", [CA, CQ], f32, isOutput=False).ap()
    wk_d = nc.declare_dram_parameter("wkT", [CA, CQ], f32, isOutput=False).ap()
    wv_d = nc.declare_dram_parameter("wvT", [CA, CV], f32, isOutput=False).ap()
    out_d = nc.declare_dram_parameter("out", [CIN, N], f32, isOutput=True).ap()

    groups = []
    j0 = 0
    while j0 < NJT:
        groups.append((j0, min(GS, NJT - j0)))
        j0 += GS

    with tile.TileContext(nc) as tc, ExitStack() as ctx:
        const = ctx.enter_context(tc.tile_pool(name="const", bufs=1))
        sb = ctx.enter_context(tc.tile_pool(name="sb", bufs=1))
        sb2 = ctx.enter_context(tc.tile_pool(name="sb2", bufs=2))
        psum = ctx.enter_context(tc.tile_pool(name="psum", bufs=2, space="PSUM"))

        wq_s = const.tile([CA, CQ], f32)
        nc.sync.dma_start(out=wq_s, in_=wq_d)
        wk_s = const.tile([CA, CQ], f32)
        nc.sync.dma_start(out=wk_s, in_=wk_d)
        wv_s = const.tile([CA, CV], f32)
        nc.sync.dma_start(out=wv_s, in_=wv_d)
        gones = const.tile([1, CIN], f32)
        nc.vector.memset(gones, gamma)

        xs = sb.tile([CA, N], f32)
        for t in range(NIT):
            sl = slice(t * IT, (t + 1) * IT)
            nc.sync.dma_start(out=xs[:, sl], in_=xa_d[:, sl])

        qs = sb.tile([CQ, N], f32)
        ks = sb.tile([CQ, N], f32)
        vts = sb.tile([128, NJT, CV], f32)

        # 1x1 convs: q/k ([6,N]) and vT chunks ([128,65] per j-chunk)
        for t in range(NIT):
            sl = slice(t * IT, (t + 1) * IT)
            pq = psum.tile([CQ, IT], f32, tag="pe")
            nc.tensor.matmul(pq, wq_s, xs[:, sl], start=True, stop=True)
            nc.vector.tensor_copy(qs[:, sl], pq)
            pk = psum.tile([CQ, IT], f32, tag="pe")
            nc.tensor.matmul(pk, wk_s, xs[:, sl], start=True, stop=True)
            nc.vector.tensor_copy(ks[:, sl], pk)
        for j in range(NJT):
            jr = min(JT, N - j * JT)
            sl = slice(j * JT, j * JT + jr)
            pv = psum.tile([128, CV], f32, tag="pe")
            nc.tensor.matmul(pv[:jr, :], xs[:, sl], wv_s, start=True, stop=True)
            nc.vector.tensor_copy(vts[:jr, j, :], pv[:jr, :])

        for t in range(NIT):
            isl = slice(t * IT, (t + 1) * IT)
            pu = psum.tile([CV, IT], f32, tag="pu")
            for (g0, gs) in groups:
                pe = psum.tile([128, GS, 512], f32, tag="pe")
                exps = sb2.tile([128, GS, IT], f32, tag="exps")
                pr = 128
                for jj in range(gs):
                    j = g0 + jj
                    jr = min(JT, N - j * JT)
                    pr = min(pr, jr)
                    nc.tensor.matmul(
                        pe[:jr, jj, :IT],
                        ks[:, j * JT : j * JT + jr],
                        qs[:, isl],
                        start=True,
                        stop=True,
                    )
                nc.scalar.activation(
                    out=exps[:pr, :gs, :],
                    in_=pe[:pr, :gs, :IT],
                    func=mybir.ActivationFunctionType.Exp,
                )
                for jj in range(gs):
                    j = g0 + jj
                    jr = min(JT, N - j * JT)
                    nc.tensor.matmul(
                        pu,
                        vts[:jr, j, :],
                        exps[:jr, jj, :],
                        start=(j == 0),
                        stop=(j == NJT - 1),
                    )
            # epilogue for this i-tile: out = gamma*U/Z + x
            usb = sb2.tile([CV, IT], f32, tag="usb")
            nc.vector.tensor_copy(usb, pu)
            rz = sb2.tile([1, IT], f32, tag="rz")
            nc.vector.reciprocal(rz, usb[CV - 1 : CV, :])
            zb = psum.tile([CIN, IT], f32, tag="pu")
            nc.tensor.matmul(zb, gones, rz, start=True, stop=True)
            osb = sb2.tile([CIN, IT], f32, tag="osb")
            nc.vector.tensor_mul(osb, usb[:CIN, :], zb)
            nc.vector.tensor_add(osb, osb, xs[:CIN, isl])
            nc.sync.dma_start(out=out_d[:, isl], in_=osb)

    nc.compile()
    return nc


def get_nc(gamma: float):
    key = float(gamma)
    if key not in _CACHE:
        _CACHE[key] = _build_nc(key)
    return _CACHE[key]


def host_prep(x, wq, bq, wk, bk, wv, bv):
    x2 = np.ascontiguousarray(
        np.asarray(x, np.float32).reshape(x.shape[0], CIN, N)
    )
    xaug = np.concatenate([x2, np.ones((x.shape[0], 1, N), np.float32)], axis=1)
    wqT = np.ascontiguousarray(
        np.concatenate([np.asarray(wq, np.float32).T, np.asarray(bq, np.float32)[None, :]], 0)
    )
    wkT = np.ascontiguousarray(
        np.concatenate([np.asarray(wk, np.float32).T, np.asarray(bk, np.float32)[None, :]], 0)
    )
    wvT = np.zeros((CA, CV), np.float32)
    wvT[:CIN, :CIN] = np.asarray(wv, np.float32).T
    wvT[CIN, :CIN] = np.asarray(bv, np.float32)
    wvT[CIN, CV - 1] = 1.0
    return xaug, wqT, wkT, wvT


def run_spmd(nc, in_maps, **kwargs):
    from concourse.bass_utils import run_bass_kernel_spmd

    return run_bass_kernel_spmd(nc, in_maps, list(range(B)), **kwargs)


def kernel(x, wq, bq, wk, bk, wv, bv, gamma):
    gamma_f = float(np.asarray(gamma, np.float32).reshape(-1)[0])
    nc = get_nc(gamma_f)
    xaug, wqT, wkT, wvT = host_prep(x, wq, bq, wk, bk, wv, bv)
    in_maps = [
        {"xa": xaug[b], "wqT": wqT, "wkT": wkT, "wvT": wvT} for b in range(B)
    ]
    res = run_spmd(nc, in_maps)
    out = np.stack([np.asarray(res.results[b]["out"]) for b in range(B)])
    return out.reshape(B, C, T, H, W).astype(np.float32)


# revision 4
# speedup vs baseline: 1.9140x; 1.9140x over previous
"""DANet PAM attention (B=8, Cin=48, N=3136) on 8 Trainium2 NeuronCores.

Sharding: data-parallel over batch — one batch per core, zero communication.

Per-core program (one batch, all fp32):
  x2 = x[b] viewed as [48, 3136]; augmented with a ones row -> xa [49, 3136].
  q = wqT.T @ xa  [6, N]        (bias folded in via the ones row)
  k = wkT.T @ xa  [6, N]
  vT chunks = xa[:, jc].T @ wvT  [128, 65]  (col 64 of wvT = ones -> vT col 64
                                             is all-ones, used to get Z)
  Energy is computed TRANSPOSED: eT[j, i] = k[:,j]^T q[:,i], j on partitions,
  so that the V-weighted sum needs no on-chip transpose and the softmax
  denominator Z[i] = sum_j exp(eT[j,i]) falls out of the same matmul as U via
  vT's ones column:
      U[c, i] (+ Z at row 64) = sum_j vT[j, c] * exp(eT[j, i])
  Softmax max-subtraction is skipped: |energy| <~ 30, exp fits fp32 easily and
  the ratio exp(e)/Z is mathematically identical.
  out = gamma * U / Z + x2  (gamma baked in as the PE-broadcast constant).

PSUM budget (8 banks): 2 x 3 banks ping-pong for energy tiles (wide grouped
ACT exp ops cut the 352-cycle per-op overhead), 2 x 1 bank for U accumulator /
Z-broadcast.
"""

from contextlib import ExitStack

import numpy as np

F32 = None  # set on first build (concourse imported lazily)

B, C, T, H, W = 8, 3, 16, 56, 56
CIN = 48         # C*T
CA = 49          # augmented input channels (ones row)
CV = 65          # vT width: cols 0-47 = channels, col 64 = ones (32-aligned Z row)
CQ = 6
N = H * W        # 3136
IT = 448         # i-tile width (free dim per PSUM bank)
NIT = N // IT    # 7
JT = 128
NJT = (N + JT - 1) // JT   # 25 (24 full + one 64-row chunk)
GS = 3           # j-chunks per exp group

_CACHE = {}


def _build_nc(gamma: float):
    import concourse.bacc as bacc
    import concourse.mybir as mybir
    import concourse.tile as tile

    f32 = mybir.dt.float32
    f32r = mybir.dt.float32r
    nc = bacc.Bacc("TRN2", target_bir_lowering=False, debug=False)

    xa_d = nc.declare_dram_parameter("xa", [CA, N], f32, isOutput=False).ap()
    wq_d = nc.declare_dram_parameter("wqT", [CA, CQ], f32, isOutput=False).ap()
    wk_d = nc.declare_dram_parameter("wkT", [CA, CQ], f32, isOutput=False).ap()
    wv_d = nc.declare_dram_parameter("wvT", [CA, CV], f32, isOutput=False).ap()
    out_d = nc.declare_dram_parameter("out", [CIN, N], f32, isOutput=True).ap()

    groups = []
    j0 = 0
    while j0 < NJT:
        groups.append((j0, min(GS, NJT - j0)))
        j0 += GS

    with tile.TileContext(nc) as tc, ExitStack() as ctx:
        const = ctx.enter_context(tc.tile_pool(name="const", bufs=1))
        sb = ctx.enter_context(tc.tile_pool(name="sb", bufs=1))
        sb2 = ctx.enter_context(tc.tile_pool(name="sb2", bufs=2))
        psum = ctx.enter_context(tc.tile_pool(name="psum", bufs=2, space="PSUM"))

        wq_s = const.tile([CA, CQ], f32)
        nc.sync.dma_start(out=wq_s, in_=wq_d)
        wk_s = const.tile([CA, CQ], f32)
        nc.sync.dma_start(out=wk_s, in_=wk_d)
        wv_s = const.tile([CA, CV], f32)
        nc.sync.dma_start(out=wv_s, in_=wv_d)
        gones = const.tile([1, CIN], f32)
        nc.vector.memset(gones, gamma)

        xs = sb.tile([CA, N], f32)
        for t in range(NIT):
            sl = slice(t * IT, (t + 1) * IT)
            nc.sync.dma_start(out=xs[:, sl], in_=xa_d[:, sl])

        qs = sb.tile([CQ, N], f32r)
        ks = sb.tile([CQ, N], f32r)
        vts = sb.tile([128, NJT, CV], f32r)

        # 1x1 convs: q/k ([6,N]) and vT chunks ([128,65] per j-chunk)
        for t in range(NIT):
            sl = slice(t * IT, (t + 1) * IT)
            pq = psum.tile([CQ, IT], f32, tag="pe")
            nc.tensor.matmul(pq, wq_s, xs[:, sl], start=True, stop=True)
            nc.vector.tensor_copy(qs[:, sl], pq)
            pk = psum.tile([CQ, IT], f32, tag="pe")
            nc.tensor.matmul(pk, wk_s, xs[:, sl], start=True, stop=True)
            nc.vector.tensor_copy(ks[:, sl], pk)
        for j in range(NJT):
            jr = min(JT, N - j * JT)
            sl = slice(j * JT, j * JT + jr)
            pv = psum.tile([128, CV], f32, tag="pe")
            nc.tensor.matmul(pv[:jr, :], xs[:, sl], wv_s, start=True, stop=True)
            nc.vector.tensor_copy(vts[:jr, j, :], pv[:jr, :])

        for t in range(NIT):
            isl = slice(t * IT, (t + 1) * IT)
            pu = psum.tile([CV, IT], f32, tag="pu")
            for (g0, gs) in groups:
                pe = psum.tile([128, GS, 512], f32, tag="pe")
                exps = sb2.tile([128, GS, IT], f32r, tag="exps")
                pr = 128
                for jj in range(gs):
                    j = g0 + jj
                    jr = min(JT, N - j * JT)
                    pr = min(pr, jr)
                    nc.tensor.matmul(
                        pe[:jr, jj, :IT],
                        ks[:, j * JT : j * JT + jr],
                        qs[:, isl],
                        start=True,
                        stop=True,
                    )
                nc.scalar.activation(
                    out=exps[:pr, :gs, :],
                    in_=pe[:pr, :gs, :IT],
                    func=mybir.ActivationFunctionType.Exp,
                )
                for jj in range(gs):
                    j = g0 + jj
                    jr = min(JT, N - j * JT)
                    nc.tensor.matmul(
                        pu,
                        vts[:jr, j, :],
                        exps[:jr, jj, :],
                        start=(j == 0),
                        stop=(j == NJT - 1),
                    )
            # epilogue for this i-tile: out = gamma*U/Z + x
            usb = sb2.tile([CV, IT], f32, tag="usb")
            nc.vector.tensor_copy(usb, pu)
            rz = sb2.tile([1, IT], f32, tag="rz")
            nc.vector.reciprocal(rz, usb[CV - 1 : CV, :])
            zb = psum.tile([CIN, IT], f32, tag="pu")
            nc.tensor.matmul(zb, gones, rz, start=True, stop=True)
            osb = sb2.tile([CIN, IT], f32, tag="osb")
            nc.vector.tensor_mul(osb, usb[:CIN, :], zb)
            nc.vector.tensor_add(osb, osb, xs[:CIN, isl])
            nc.sync.dma_start(out=out_d[:, isl], in_=osb)

    nc.compile()
    return nc


def get_nc(gamma: float):
    key = float(gamma)
    if key not in _CACHE:
        _CACHE[key] = _build_nc(key)
    return _CACHE[key]


def host_prep(x, wq, bq, wk, bk, wv, bv):
    x2 = np.ascontiguousarray(
        np.asarray(x, np.float32).reshape(x.shape[0], CIN, N)
    )
    xaug = np.concatenate([x2, np.ones((x.shape[0], 1, N), np.float32)], axis=1)
    wqT = np.ascontiguousarray(
        np.concatenate([np.asarray(wq, np.float32).T, np.asarray(bq, np.float32)[None, :]], 0)
    )
    wkT = np.ascontiguousarray(
        np.concatenate([np.asarray(wk, np.float32).T, np.asarray(bk, np.float32)[None, :]], 0)
    )
    wvT = np.zeros((CA, CV), np.float32)
    wvT[:CIN, :CIN] = np.asarray(wv, np.float32).T
    wvT[CIN, :CIN] = np.asarray(bv, np.float32)
    wvT[CIN, CV - 1] = 1.0
    return xaug, wqT, wkT, wvT


def run_spmd(nc, in_maps, **kwargs):
    from concourse.bass_utils import run_bass_kernel_spmd

    return run_bass_kernel_spmd(nc, in_maps, list(range(B)), **kwargs)


def kernel(x, wq, bq, wk, bk, wv, bv, gamma):
    gamma_f = float(np.asarray(gamma, np.float32).reshape(-1)[0])
    nc = get_nc(gamma_f)
    xaug, wqT, wkT, wvT = host_prep(x, wq, bq, wk, bk, wv, bv)
    in_maps = [
        {"xa": xaug[b], "wqT": wqT, "wkT": wkT, "wvT": wvT} for b in range(B)
    ]
    res = run_spmd(nc, in_maps)
    out = np.stack([np.asarray(res.results[b]["out"]) for b in range(B)])
    return out.reshape(B, C, T, H, W).astype(np.float32)


# revision 7
# speedup vs baseline: 2.4291x; 1.2691x over previous
"""DANet PAM attention (B=8, Cin=48, N=3136) on 8 Trainium2 NeuronCores.

Sharding: data-parallel over batch — one batch per core, zero communication.

Per-core program (one batch, all fp32):
  x2 = x[b] viewed as [48, 3136]; augmented with a ones row -> xa [49, 3136].
  q = wqT.T @ xa  [6, N]        (bias folded in via the ones row)
  k = wkT.T @ xa  [6, N]
  vT chunks = xa[:, jc].T @ wvT  [128, 65]  (col 64 of wvT = ones -> vT col 64
                                             is all-ones, used to get Z)
  Energy is computed TRANSPOSED: eT[j, i] = k[:,j]^T q[:,i], j on partitions,
  so that the V-weighted sum needs no on-chip transpose and the softmax
  denominator Z[i] = sum_j exp(eT[j,i]) falls out of the same matmul as U via
  vT's ones column:
      U[c, i] (+ Z at row 64) = sum_j vT[j, c] * exp(eT[j, i])
  Softmax max-subtraction is skipped: |energy| <~ 30, exp fits fp32 easily and
  the ratio exp(e)/Z is mathematically identical.
  out = gamma * U / Z + x2  (gamma baked in as the PE-broadcast constant).

PSUM budget (8 banks): 2 x 3 banks ping-pong for energy tiles (wide grouped
ACT exp ops cut the 352-cycle per-op overhead), 2 x 1 bank for U accumulator /
Z-broadcast.
"""

from contextlib import ExitStack

import numpy as np

F32 = None  # set on first build (concourse imported lazily)

B, C, T, H, W = 8, 3, 16, 56, 56
CIN = 48         # C*T
CA = 49          # augmented input channels (ones row)
CV = 65          # vT width: cols 0-47 = channels, col 64 = ones (32-aligned Z row)
CQ = 6
N = H * W        # 3136
IT = 448         # i-tile width (free dim per PSUM bank)
NIT = N // IT    # 7
JT = 128
NJT = (N + JT - 1) // JT   # 25 (24 full + one 64-row chunk)
GS = 3           # j-chunks per exp group

_CACHE = {}


def _build_nc(gamma: float):
    import concourse.bacc as bacc
    import concourse.mybir as mybir
    import concourse.tile as tile

    f32 = mybir.dt.float32
    f32r = mybir.dt.float32r
    f16 = mybir.dt.float16
    nc = bacc.Bacc("TRN2", target_bir_lowering=False, debug=False)

    xa_d = nc.declare_dram_parameter("xa", [CA, N], f32, isOutput=False).ap()
    wq_d = nc.declare_dram_parameter("wqT", [CA, 70], f32, isOutput=False).ap()
    wk_d = nc.declare_dram_parameter("wkT", [CA, 70], f32, isOutput=False).ap()
    wv_d = nc.declare_dram_parameter("wvT", [CA, CV], f32, isOutput=False).ap()
    out_d = nc.declare_dram_parameter("out", [CIN, N], f32, isOutput=True).ap()

    groups = []
    j0 = 0
    while j0 < NJT:
        groups.append((j0, min(GS, NJT - j0)))
        j0 += GS

    with tile.TileContext(nc) as tc, ExitStack() as ctx:
        const = ctx.enter_context(tc.tile_pool(name="const", bufs=1))
        sb = ctx.enter_context(tc.tile_pool(name="sb", bufs=1))
        sb2 = ctx.enter_context(tc.tile_pool(name="sb2", bufs=2))
        psum = ctx.enter_context(tc.tile_pool(name="psum", bufs=2, space="PSUM"))

        wq_s = const.tile([CA, 70], f32)
        nc.sync.dma_start(out=wq_s, in_=wq_d)
        wk_s = const.tile([CA, 70], f32)
        nc.sync.dma_start(out=wk_s, in_=wk_d)
        wv_s = const.tile([CA, CV], f32)
        nc.sync.dma_start(out=wv_s, in_=wv_d)
        wqr = const.tile([CA, 70], f32r)
        nc.vector.tensor_copy(wqr, wq_s)
        wkr = const.tile([CA, 70], f32r)
        nc.vector.tensor_copy(wkr, wk_s)
        wvr = const.tile([CA, CV], f32r)
        nc.vector.tensor_copy(wvr, wv_s)
        gones = const.tile([1, CIN], f32)
        nc.vector.memset(gones, gamma)

        xs = sb.tile([CA, N], f32)
        for t in range(NIT):
            sl = slice(t * IT, (t + 1) * IT)
            nc.sync.dma_start(out=xs[:, sl], in_=xa_d[:, sl])

        xsr = sb.tile([CA, N], f32r)
        for t in range(NIT):
            sl = slice(t * IT, (t + 1) * IT)
            nc.vector.tensor_copy(xsr[:, sl], xs[:, sl])

        qs = sb.tile([70, N], f16)
        ks = sb.tile([70, N], f16)
        vts = sb.tile([128, NJT, CV], f32r)

        # 1x1 convs: q/k ([6,N]) and vT chunks ([128,65] per j-chunk)
        for t in range(NIT):
            sl = slice(t * IT, (t + 1) * IT)
            pq = psum.tile([70, IT], f32, tag="pe")
            nc.tensor.matmul(pq, wqr, xsr[:, sl], start=True, stop=True)
            nc.vector.tensor_copy(qs[:, sl], pq)
            pk = psum.tile([70, IT], f32, tag="pe")
            nc.tensor.matmul(pk, wkr, xsr[:, sl], start=True, stop=True)
            nc.vector.tensor_copy(ks[:, sl], pk)
        for j in range(NJT):
            jr = min(JT, N - j * JT)
            sl = slice(j * JT, j * JT + jr)
            pv = psum.tile([128, CV], f32, tag="pe")
            nc.tensor.matmul(pv[:jr, :], xs[:, sl], wv_s, start=True, stop=True)
            nc.vector.tensor_copy(vts[:jr, j, :], pv[:jr, :])

        for t in range(NIT):
            isl = slice(t * IT, (t + 1) * IT)
            pu = psum.tile([CV, IT], f32, tag="pu")
            for (g0, gs) in groups:
                pe = psum.tile([128, GS, 512], f32, tag="pe")
                exps = sb2.tile([128, GS, IT], f32r, tag="exps")
                pr = 128
                for jj in range(gs):
                    j = g0 + jj
                    jr = min(JT, N - j * JT)
                    pr = min(pr, jr)
                    rb = 32 * jj
                    nc.tensor.matmul(
                        pe[:jr, jj, :IT],
                        ks[rb : rb + CQ, j * JT : j * JT + jr],
                        qs[rb : rb + CQ, isl],
                        start=True,
                        stop=True,
                        tile_position=(rb, 0),
                    )
                nc.scalar.activation(
                    out=exps[:pr, :gs, :],
                    in_=pe[:pr, :gs, :IT],
                    func=mybir.ActivationFunctionType.Exp,
                )
                for jj in range(gs):
                    j = g0 + jj
                    jr = min(JT, N - j * JT)
                    nc.tensor.matmul(
                        pu,
                        vts[:jr, j, :],
                        exps[:jr, jj, :],
                        start=(j == 0),
                        stop=(j == NJT - 1),
                    )
            # epilogue for this i-tile: out = gamma*U/Z + x
            usb = sb2.tile([CV, IT], f32, tag="usb")
            nc.vector.tensor_copy(usb, pu)
            rz = sb2.tile([1, IT], f32, tag="rz")
            nc.vector.reciprocal(rz, usb[CV - 1 : CV, :])
            zb = psum.tile([CIN, IT], f32, tag="pu")
            nc.tensor.matmul(zb, gones, rz, start=True, stop=True)
            osb = sb2.tile([CIN, IT], f32, tag="osb")
            nc.vector.tensor_mul(osb, usb[:CIN, :], zb)
            nc.vector.tensor_add(osb, osb, xs[:CIN, isl])
            nc.sync.dma_start(out=out_d[:, isl], in_=osb)

    nc.compile()
    return nc


def get_nc(gamma: float):
    key = float(gamma)
    if key not in _CACHE:
        _CACHE[key] = _build_nc(key)
    return _CACHE[key]


def host_prep(x, wq, bq, wk, bk, wv, bv):
    x2 = np.ascontiguousarray(
        np.asarray(x, np.float32).reshape(x.shape[0], CIN, N)
    )
    xaug = np.concatenate([x2, np.ones((x.shape[0], 1, N), np.float32)], axis=1)
    wqT = np.concatenate([np.asarray(wq, np.float32).T, np.asarray(bq, np.float32)[None, :]], 0)
    wkT = np.concatenate([np.asarray(wk, np.float32).T, np.asarray(bk, np.float32)[None, :]], 0)
    # replicate q/k weight columns at offsets 0/32/64 so the energy matmuls can
    # be row-packed 3x via tile_position
    wq3 = np.zeros((CA, 70), np.float32)
    wk3 = np.zeros((CA, 70), np.float32)
    for g in range(3):
        wq3[:, 32 * g : 32 * g + CQ] = wqT
        wk3[:, 32 * g : 32 * g + CQ] = wkT
    wqT, wkT = np.ascontiguousarray(wq3), np.ascontiguousarray(wk3)
    wvT = np.zeros((CA, CV), np.float32)
    wvT[:CIN, :CIN] = np.asarray(wv, np.float32).T
    wvT[CIN, :CIN] = np.asarray(bv, np.float32)
    wvT[CIN, CV - 1] = 1.0
    return xaug, wqT, wkT, wvT


def run_spmd(nc, in_maps, **kwargs):
    from concourse.bass_utils import run_bass_kernel_spmd

    return run_bass_kernel_spmd(nc, in_maps, list(range(B)), **kwargs)


def kernel(x, wq, bq, wk, bk, wv, bv, gamma):
    gamma_f = float(np.asarray(gamma, np.float32).reshape(-1)[0])
    nc = get_nc(gamma_f)
    xaug, wqT, wkT, wvT = host_prep(x, wq, bq, wk, bk, wv, bv)
    in_maps = [
        {"xa": xaug[b], "wqT": wqT, "wkT": wkT, "wvT": wvT} for b in range(B)
    ]
    res = run_spmd(nc, in_maps)
    out = np.stack([np.asarray(res.results[b]["out"]) for b in range(B)])
    return out.reshape(B, C, T, H, W).astype(np.float32)


# revision 8
# speedup vs baseline: 2.5418x; 1.0464x over previous
"""DANet PAM attention (B=8, Cin=48, N=3136) on 8 Trainium2 NeuronCores.

Sharding: data-parallel over batch — one batch per core, zero communication.

Per-core program (one batch, all fp32):
  x2 = x[b] viewed as [48, 3136]; augmented with a ones row -> xa [49, 3136].
  q = wqT.T @ xa  [6, N]        (bias folded in via the ones row)
  k = wkT.T @ xa  [6, N]
  vT chunks = xa[:, jc].T @ wvT  [128, 65]  (col 64 of wvT = ones -> vT col 64
                                             is all-ones, used to get Z)
  Energy is computed TRANSPOSED: eT[j, i] = k[:,j]^T q[:,i], j on partitions,
  so that the V-weighted sum needs no on-chip transpose and the softmax
  denominator Z[i] = sum_j exp(eT[j,i]) falls out of the same matmul as U via
  vT's ones column:
      U[c, i] (+ Z at row 64) = sum_j vT[j, c] * exp(eT[j, i])
  Softmax max-subtraction is skipped: |energy| <~ 30, exp fits fp32 easily and
  the ratio exp(e)/Z is mathematically identical.
  out = gamma * U / Z + x2  (gamma baked in as the PE-broadcast constant).

PSUM budget (8 banks): 2 x 3 banks ping-pong for energy tiles (wide grouped
ACT exp ops cut the 352-cycle per-op overhead), 2 x 1 bank for U accumulator /
Z-broadcast.
"""

from contextlib import ExitStack

import numpy as np

F32 = None  # set on first build (concourse imported lazily)

B, C, T, H, W = 8, 3, 16, 56, 56
CIN = 48         # C*T
CA = 49          # augmented input channels (ones row)
CV = 65          # vT width: cols 0-47 = channels, col 64 = ones (32-aligned Z row)
CQ = 6
N = H * W        # 3136
IT = 448         # i-tile width (free dim per PSUM bank)
NIT = N // IT    # 7
JT = 128
NJT = (N + JT - 1) // JT   # 25 (24 full + one 64-row chunk)
GS = 3           # j-chunks per exp group

_CACHE = {}


def _build_nc(gamma: float):
    import concourse.bacc as bacc
    import concourse.mybir as mybir
    import concourse.tile as tile

    f32 = mybir.dt.float32
    f32r = mybir.dt.float32r
    f16 = mybir.dt.float16
    bf16 = mybir.dt.bfloat16
    nc = bacc.Bacc("TRN2", target_bir_lowering=False, debug=False)

    xa_d = nc.declare_dram_parameter("xa", [CA, N], f32, isOutput=False).ap()
    wq_d = nc.declare_dram_parameter("wqT", [CA, 70], f32, isOutput=False).ap()
    wk_d = nc.declare_dram_parameter("wkT", [CA, 70], f32, isOutput=False).ap()
    wv_d = nc.declare_dram_parameter("wvT", [CA, CV], f32, isOutput=False).ap()
    out_d = nc.declare_dram_parameter("out", [CIN, N], f32, isOutput=True).ap()

    groups = []
    j0 = 0
    while j0 < NJT:
        groups.append((j0, min(GS, NJT - j0)))
        j0 += GS

    with tile.TileContext(nc) as tc, ExitStack() as ctx:
        const = ctx.enter_context(tc.tile_pool(name="const", bufs=1))
        sb = ctx.enter_context(tc.tile_pool(name="sb", bufs=1))
        sb2 = ctx.enter_context(tc.tile_pool(name="sb2", bufs=2))
        psum = ctx.enter_context(tc.tile_pool(name="psum", bufs=2, space="PSUM"))

        wq_s = const.tile([CA, 70], f32)
        nc.sync.dma_start(out=wq_s, in_=wq_d)
        wk_s = const.tile([CA, 70], f32)
        nc.sync.dma_start(out=wk_s, in_=wk_d)
        wv_s = const.tile([CA, CV], f32)
        nc.sync.dma_start(out=wv_s, in_=wv_d)
        wqr = const.tile([CA, 70], f32r)
        nc.vector.tensor_copy(wqr, wq_s)
        wkr = const.tile([CA, 70], f32r)
        nc.vector.tensor_copy(wkr, wk_s)
        wvr = const.tile([CA, CV], f32r)
        nc.vector.tensor_copy(wvr, wv_s)
        gones = const.tile([1, CIN], f32)
        nc.vector.memset(gones, gamma)
        gonesr = const.tile([1, CIN], f32r)
        nc.vector.tensor_copy(gonesr, gones)

        xs = sb.tile([CA, N], f32)
        for t in range(NIT):
            sl = slice(t * IT, (t + 1) * IT)
            nc.sync.dma_start(out=xs[:, sl], in_=xa_d[:, sl])

        xsr = sb.tile([CA, N], f32r)
        for t in range(NIT):
            sl = slice(t * IT, (t + 1) * IT)
            nc.vector.tensor_copy(xsr[:, sl], xs[:, sl])

        qs = sb.tile([70, N], f16)
        ks = sb.tile([70, N], f16)
        vts = sb.tile([128, NJT, CV], bf16)

        # 1x1 convs: q/k ([6,N]) and vT chunks ([128,65] per j-chunk)
        for t in range(NIT):
            sl = slice(t * IT, (t + 1) * IT)
            pq = psum.tile([70, IT], f32, tag="pe")
            nc.tensor.matmul(pq, wqr, xsr[:, sl], start=True, stop=True)
            nc.vector.tensor_copy(qs[:, sl], pq)
            pk = psum.tile([70, IT], f32, tag="pe")
            nc.tensor.matmul(pk, wkr, xsr[:, sl], start=True, stop=True)
            nc.vector.tensor_copy(ks[:, sl], pk)
        for j in range(NJT):
            jr = min(JT, N - j * JT)
            sl = slice(j * JT, j * JT + jr)
            pv = psum.tile([128, CV], f32, tag="pe")
            nc.tensor.matmul(pv[:jr, :], xs[:, sl], wv_s, start=True, stop=True)
            nc.vector.tensor_copy(vts[:jr, j, :], pv[:jr, :])

        for t in range(NIT):
            isl = slice(t * IT, (t + 1) * IT)
            pu = psum.tile([CV, IT], f32, tag="pu")
            for (g0, gs) in groups:
                pe = psum.tile([128, GS, 512], f32, tag="pe")
                exps = sb2.tile([128, GS, IT], bf16, tag="exps")
                pr = 128
                for jj in range(gs):
                    j = g0 + jj
                    jr = min(JT, N - j * JT)
                    pr = min(pr, jr)
                    rb = 32 * jj
                    nc.tensor.matmul(
                        pe[:jr, jj, :IT],
                        ks[rb : rb + CQ, j * JT : j * JT + jr],
                        qs[rb : rb + CQ, isl],
                        start=True,
                        stop=True,
                        tile_position=(rb, 0),
                    )
                nc.scalar.activation(
                    out=exps[:pr, :gs, :],
                    in_=pe[:pr, :gs, :IT],
                    func=mybir.ActivationFunctionType.Exp,
                )
                for jj in range(gs):
                    j = g0 + jj
                    jr = min(JT, N - j * JT)
                    nc.tensor.matmul(
                        pu,
                        vts[:jr, j, :],
                        exps[:jr, jj, :],
                        start=(j == 0),
                        stop=(j == NJT - 1),
                    )
            # epilogue for this i-tile: out = gamma*U/Z + x
            usb = sb2.tile([CV, IT], f32, tag="usb")
            nc.vector.tensor_copy(usb, pu)
            rz = sb2.tile([1, IT], f32, tag="rz")
            nc.vector.reciprocal(rz, usb[CV - 1 : CV, :])
            rzr = sb2.tile([1, IT], f32r, tag="rzr")
            nc.vector.tensor_copy(rzr, rz)
            zb = psum.tile([CIN, IT], f32, tag="pu")
            nc.tensor.matmul(zb, gonesr, rzr, start=True, stop=True)
            osb = sb2.tile([CIN, IT], f32, tag="osb")
            nc.vector.tensor_mul(osb, usb[:CIN, :], zb)
            nc.vector.tensor_add(osb, osb, xs[:CIN, isl])
            nc.sync.dma_start(out=out_d[:, isl], in_=osb)

    nc.compile()
    return nc


def get_nc(gamma: float):
    key = float(gamma)
    if key not in _CACHE:
        _CACHE[key] = _build_nc(key)
    return _CACHE[key]


def host_prep(x, wq, bq, wk, bk, wv, bv):
    x2 = np.ascontiguousarray(
        np.asarray(x, np.float32).reshape(x.shape[0], CIN, N)
    )
    xaug = np.concatenate([x2, np.ones((x.shape[0], 1, N), np.float32)], axis=1)
    wqT = np.concatenate([np.asarray(wq, np.float32).T, np.asarray(bq, np.float32)[None, :]], 0)
    wkT = np.concatenate([np.asarray(wk, np.float32).T, np.asarray(bk, np.float32)[None, :]], 0)
    # replicate q/k weight columns at offsets 0/32/64 so the energy matmuls can
    # be row-packed 3x via tile_position
    wq3 = np.zeros((CA, 70), np.float32)
    wk3 = np.zeros((CA, 70), np.float32)
    for g in range(3):
        wq3[:, 32 * g : 32 * g + CQ] = wqT
        wk3[:, 32 * g : 32 * g + CQ] = wkT
    wqT, wkT = np.ascontiguousarray(wq3), np.ascontiguousarray(wk3)
    wvT = np.zeros((CA, CV), np.float32)
    wvT[:CIN, :CIN] = np.asarray(wv, np.float32).T
    wvT[CIN, :CIN] = np.asarray(bv, np.float32)
    wvT[CIN, CV - 1] = 1.0
    return xaug, wqT, wkT, wvT


def run_spmd(nc, in_maps, **kwargs):
    from concourse.bass_utils import run_bass_kernel_spmd

    return run_bass_kernel_spmd(nc, in_maps, list(range(B)), **kwargs)


def kernel(x, wq, bq, wk, bk, wv, bv, gamma):
    gamma_f = float(np.asarray(gamma, np.float32).reshape(-1)[0])
    nc = get_nc(gamma_f)
    xaug, wqT, wkT, wvT = host_prep(x, wq, bq, wk, bk, wv, bv)
    in_maps = [
        {"xa": xaug[b], "wqT": wqT, "wkT": wkT, "wvT": wvT} for b in range(B)
    ]
    res = run_spmd(nc, in_maps)
    out = np.stack([np.asarray(res.results[b]["out"]) for b in range(B)])
    return out.reshape(B, C, T, H, W).astype(np.float32)
